# revision 33
# baseline (speedup 1.0000x reference)
"""Cross-attention Trainium2 kernel (8 NeuronCores), v8.

Sharding: batch (2) x head-groups (4 groups of 4 heads) = 8 shards.
Each core computes q/k/v projections for its 4 heads (256 cols of
Wq/Wk/Wv), attention for those heads, and a partial out-projection
through its 256 rows of Wo.  The host sums the 4 partial outputs per
batch and adds the bv @ Wo + bo correction.

v7 layout/schedule (HW-calibrated: MM N=512 ~280ns accumulating /
~230ns one-shot K=128 / ~275ns K=64-alternating; ACT exp [128,1024]
~1350ns; consecutive matmuls into ALTERNATING psum banks pipeline
measurably better than same-bank chains, hence the interleaved
projection/out_proj chains and the half-outer score order):
  - scores are K=128 ONE-SHOT matmuls: kT is stored split into kTe
    (even head rows 0-63, zeros 64-127) and kTo (zeros 0-63, odd head
    rows 64-127) so each score matmul contracts the full 128 partitions
    (the zero rows annihilate the other head's q contribution).
  - odd-head normalization writes aTw[64:128] directly with a
    partition-shifted DVE multiply (out partitions 64-127, ins 0-63) --
    no identity matmul, no extra copy.
  - phases 1-3 are FUSED sweeps (both sq halves accumulate per skc in 4
    PSUM banks: atp x2 + pp x2, stp x2 for the last phase), so each exp
    tile is fully consumed at its skc step; the next pair's scores are
    emitted 1-per-skc inside the sweep, keeping ACT fed through the
    norm + out_proj stretches after each sweep.  Phase 0 stays split in
    two half-sweeps because proj_v needs the pp PSUM banks.
  - norm chains never touch the PE queue; held-back score emissions
    cover the PE while norms run on DVE/Pool.
  - out_proj batches run after the norms that complete their window:
    (0-7) after phase 1, (8-11)/(12-15) inside phase 3's tail.
  - ~1/5 of exp tiles go to DVE via Schraudolph bitcast exp.
  - input DMAs are split into dc-halves so the first projection
    matmuls start ~3us earlier (subtile deps gate per-half).
"""

import numpy as np

import concourse.bass as bass
import concourse.mybir as mybir
import concourse.tile as tile
from concourse import bacc

B, SQ, SK, D, H, HS = 2, 2048, 2048, 1024, 16, 64
SCALE = HS ** -0.5
NCORES = 8
HG = 4            # heads per core
DG = HG * HS      # 256 projection cols per core

F32 = mybir.dt.float32
F16 = mybir.dt.float16
BF16 = mybir.dt.bfloat16


def build_program(loop_iters: int = 0):
    """Build the per-core SPMD Bass program."""
    nc = bacc.Bacc(None, target_bir_lowering=False, debug=False,
                   num_devices=NCORES)
    x_d = nc.dram_tensor("xT", [D, SQ], F16, kind="ExternalInput")
    c_d = nc.dram_tensor("cT", [D, SK], F16, kind="ExternalInput")
    wq_d = nc.dram_tensor("wq", [D, DG], F16, kind="ExternalInput")
    wk_d = nc.dram_tensor("wk", [D, DG], F16, kind="ExternalInput")
    wv_d = nc.dram_tensor("wv", [D, DG], F16, kind="ExternalInput")
    wo_d = nc.dram_tensor("wo", [DG, D], F16, kind="ExternalInput")
    bq_d = nc.dram_tensor("bq", [DG], F32, kind="ExternalInput")
    bk_d = nc.dram_tensor("bk", [DG], F32, kind="ExternalInput")
    out_d = nc.dram_tensor("out", [SQ, D], BF16, kind="ExternalOutput")

    with tile.TileContext(nc) as tc:
        with (
            tc.tile_pool(name="const", bufs=1) as cp,
            tc.tile_pool(name="persist", bufs=1) as psb,
            tc.tile_pool(name="xw", bufs=5) as xwp,
            tc.tile_pool(name="expp", bufs=42) as ep,
            tc.tile_pool(name="fin", bufs=4) as fpool,
            tc.tile_pool(name="outp", bufs=2) as opool,
            tc.tile_pool(name="pp", bufs=2, space="PSUM") as pp,
            tc.tile_pool(name="stp", bufs=2, space="PSUM") as stp,
            tc.tile_pool(name="atp", bufs=2, space="PSUM") as atp,
        ):
            import contextlib

            wq_sb = cp.tile([128, 8, DG], F16, tag="wq")
            wk_sb = cp.tile([128, 8, DG], F16, tag="wk")
            wv_sb = cp.tile([128, 8, DG], F16, tag="wv")
            wo_sb = cp.tile([128, 2, D], F16, tag="wo")
            bq_sb = cp.tile([128, 2], F32, tag="bq")
            bk_sb = cp.tile([128, 2], F32, tag="bk")

            # persistent activations.
            # qT: [128, 2, SQ] head pairs stacked (even p0-63, odd p64-127)
            # kTe/kTo: zero-padded split so score matmuls contract K=128.
            qT = psb.tile([128, 2, SQ], F16, tag="qT", name="qT")
            kTe = psb.tile([128, 2, SK], F16, tag="kTe", name="kTe")
            kTo = psb.tile([128, 2, SK], F16, tag="kTo", name="kTo")
            # v natural: [sk-chunk part, skc, head, 64+ones]
            vA = psb.tile([128, 16, HG, 68], F16, tag="vA", name="vA")
            # attn^T per sq-window: [pair-stacked head dim, pair, sq]
            aTw = [psb.tile([128, 2, 1024], F16, tag=f"aTw{s}", name=f"aTw{s}")
                   for s in range(2)]

            # zero/ones fills: once, outside the timing loop (in the real
            # single-shot execution they overlap the input DMAs; nothing
            # inside an iteration dirties the constant regions)
            nc.vector.memset(vA[:], 1.0)
            nc.vector.memset(kTe[64:128, :, :], 0.0)
            nc.vector.memset(kTo[0:64, :, :], 0.0)

            loop_ctx = tc.For_i(0, loop_iters, 1) if loop_iters else contextlib.nullcontext()
            loop_ctx.__enter__()

            def load_weights_qx():
                # split into dc-halves: the first projection matmuls only
                # need dc 0-3, so they start ~2us earlier (subtile deps)
                wqv = wq_d[:].rearrange("(c p) n -> p c n", p=128)
                nc.sync.dma_start(out=wq_sb[:, 0:4, :], in_=wqv[:, 0:4, :])
                nc.sync.dma_start(out=bq_sb, in_=bq_d[:].rearrange("(c p) -> p c", p=128))
                nc.sync.dma_start(out=wq_sb[:, 4:8, :], in_=wqv[:, 4:8, :])

            def load_weights_k():
                wkv = wk_d[:].rearrange("(c p) n -> p c n", p=128)
                nc.sync.dma_start(out=wk_sb[:, 0:4, :], in_=wkv[:, 0:4, :])
                nc.sync.dma_start(out=bk_sb, in_=bk_d[:].rearrange("(c p) -> p c", p=128))
                nc.sync.dma_start(out=wk_sb[:, 4:8, :], in_=wkv[:, 4:8, :])

            def load_weights_v():
                nc.sync.dma_start(out=wv_sb, in_=wv_d[:].rearrange("(c p) n -> p c n", p=128))

            def load_weights_o():
                nc.sync.dma_start(out=wo_sb, in_=wo_d[:].rearrange("(c p) n -> p c n", p=128))

            cws = {}

            def proj_x(w, cs, dma=False, after_dma=None):
                if dma:
                    xw = xwp.tile([128, 8, 512], F16, tag="xw")
                    cws[("x", w)] = xw
                    xv = (x_d[:, w * 512:(w + 1) * 512]
                          .rearrange("(c p) s -> p c s", p=128))
                    nc.sync.dma_start(out=xw[:, 0:4, :], in_=xv[:, 0:4, :])
                    if after_dma is not None:
                        after_dma()
                    nc.sync.dma_start(out=xw[:, 4:8, :], in_=xv[:, 4:8, :])
                xw = cws[("x", w)]
                # interleave the chunks' accumulation chains MM-by-MM so
                # consecutive matmuls target alternating PSUM banks
                pqs = [pp.tile([128, 512], F32, tag="pp", name=f"pq{c}")
                       for c in cs]
                for dc in range(8):
                    for j, c in enumerate(cs):
                        nc.tensor.matmul(
                            pqs[j],
                            (wq_sb[:, dc, c * 128:(c + 1) * 128]),
                            (xw[:, dc, :]),
                            start=(dc == 0), stop=(dc == 7),
                        )
                for j, c in enumerate(cs):
                    nc.vector.tensor_scalar_add(
                        qT[:, c, w * 512:(w + 1) * 512], pqs[j],
                        bq_sb[:, c:c + 1])
                if cs[-1] == 1:
                    del cws[("x", w)]

            def proj_k(w, cs, dma=False, after_dma=None):
                if dma:
                    cw = xwp.tile([128, 8, 512], F16, tag="xw")
                    cws[("c", w)] = cw
                    cv = (c_d[:, w * 512:(w + 1) * 512]
                          .rearrange("(c p) s -> p c s", p=128))
                    nc.sync.dma_start(out=cw[:, 0:4, :], in_=cv[:, 0:4, :])
                    if after_dma is not None:
                        after_dma()
                    nc.sync.dma_start(out=cw[:, 4:8, :], in_=cv[:, 4:8, :])
                cw = cws[("c", w)]
                pks = [pp.tile([128, 512], F32, tag="pp", name=f"pk{c}")
                       for c in cs]
                for dc in range(8):
                    for j, c in enumerate(cs):
                        nc.tensor.matmul(
                            pks[j],
                            (wk_sb[:, dc, c * 128:(c + 1) * 128]),
                            (cw[:, dc, :]),
                            start=(dc == 0), stop=(dc == 7),
                        )
                sl = slice(w * 512, (w + 1) * 512)
                for j, c in enumerate(cs):
                    nc.vector.tensor_scalar_add(
                        kTe[0:64, c, sl], pks[j][0:64, :],
                        bk_sb[0:64, c:c + 1])
                    nc.vector.tensor_scalar_add(
                        kTo[64:128, c, sl], pks[j][64:128, :],
                        bk_sb[64:128, c:c + 1])

            def proj_v(w, s4s):
                cw = cws[("c", w)]
                pvs = [pp.tile([128, 512], F32, tag="pp", name=f"pv{s4}")
                       for s4 in s4s]
                for dc in range(8):
                    for j, s4 in enumerate(s4s):
                        nc.tensor.matmul(
                            pvs[j][:, :DG],
                            (cw[:, dc, s4 * 128:(s4 + 1) * 128]),
                            (wv_sb[:, dc, :]),
                            start=(dc == 0), stop=(dc == 7),
                        )
                for j, s4 in enumerate(s4s):
                    nc.vector.tensor_copy(
                        vA[:, w * 4 + s4, :, 0:64],
                        pvs[j][:, :DG].rearrange("p (h e) -> p h e", e=64),
                    )
                if s4s[-1] == 3:
                    del cws[("c", w)]

            # scores + exp for head pair t, sq window sqw, one sk chunk.
            # K=128 one-shot matmuls against the zero-padded kTe/kTo.
            # Schraudolph exp for DVE-offloaded tiles:
            # bitcast_f32(round(A*score + B)) ~= exp(SCALE*score), ~2% RMS
            SCH_A = SCALE * 1.4426950408889634 * 8388608.0
            SCH_B = 1064866805.0
            I32 = mybir.dt.int32

            def emit_se(t, sqw, skc, dve=False):
                sts = [stp.tile([128, 1024], F32, tag="st", name=f"st{p}")
                       for p in range(2)]
                # half outer / par inner: consecutive matmuls alternate
                # PSUM tiles (banks), which pipelines better on HW
                for half in range(2):
                    for par, kt in ((0, kTe), (1, kTo)):
                        nc.tensor.matmul(
                            sts[par][:, half * 512:(half + 1) * 512],
                            (kt[:, t, skc * 128:(skc + 1) * 128]),
                            (qT[:, t,
                                sqw * 1024 + half * 512:
                                sqw * 1024 + (half + 1) * 512]),
                            start=True, stop=True,
                        )
                exs = []
                for par in range(2):
                    ex = ep.tile([128, 1024], F16, tag="ex")
                    if dve:
                        nc.vector.tensor_scalar(
                            sts[par][:].bitcast(I32), sts[par][:],
                            SCH_A, SCH_B,
                            mybir.AluOpType.mult, mybir.AluOpType.add)
                        nc.vector.tensor_copy(ex, sts[par][:])
                    else:
                        nc.scalar.activation(
                            ex, sts[par], mybir.ActivationFunctionType.Exp,
                            scale=SCALE)
                    exs.append(ex)
                return exs

            # one attention accumulation matmul: stationary v(+ones),
            # moving a 512-wide half of the exp tile
            def mm_at(at, ex, h, skc, half):
                nc.tensor.matmul(
                    at[0:68, :],
                    vA[:, skc, h, :],
                    ex[:, half * 512:(half + 1) * 512],
                    start=(skc == 0), stop=(skc == 15),
                )

            # normalize one head's attn^T half into aTw.  Odd heads (par 1)
            # use a partition-shifted DVE multiply (out 64-127, ins 0-63).
            def norm(t, sqw, par, half, at):
                rcrow = fpool.tile([1, 512], F32, tag="rcrow")
                nc.vector.reciprocal(rcrow, at[64:65, :])
                rc = fpool.tile([64, 512], F32, tag="rc")
                nc.gpsimd.partition_broadcast(rc, rcrow)
                dst = aTw[sqw][64 * par:64 * par + 64, t,
                               half * 512:(half + 1) * 512]
                nc.vector.tensor_mul(dst, at[0:64, :], rc)

            # partial out-projection for one 128-row sq chunk
            def emit_out_proj(sqc, use_act=False, po_pool=None):
                ot = opool.tile([128, D], BF16, tag="ot")
                sqw, c8 = sqc // 8, sqc % 8
                opl, optag = po_pool or (pp, "pp")
                pos = [opl.tile([128, 512], F32, tag=optag, name=f"po{n2}")
                       for n2 in range(2)]
                for kc in range(2):
                    for n2 in range(2):
                        nc.tensor.matmul(
                            pos[n2],
                            (aTw[sqw][:, kc, c8 * 128:(c8 + 1) * 128]),
                            (wo_sb[:, kc, n2 * 512:(n2 + 1) * 512]),
                            start=(kc == 0), stop=(kc == 1),
                        )
                for n2 in range(2):
                    if use_act and n2 == 1:
                        nc.scalar.copy(ot[:, n2 * 512:(n2 + 1) * 512],
                                       pos[n2])
                    else:
                        nc.vector.tensor_copy(
                            ot[:, n2 * 512:(n2 + 1) * 512], pos[n2])
                nc.sync.dma_start(
                    out=out_d[sqc * 128:(sqc + 1) * 128, :], in_=ot)

            # ---- prologue: pair-0 projection columns first, with the
            # first pair's scores spread between projection chunks so ACT
            # starts early and stays fed
            P = [(0, 0), (1, 0), (0, 1), (1, 1)]
            e = {}
            se0 = []
            proj_x(0, [0], dma=True, after_dma=load_weights_qx)
            proj_k(0, [0], dma=True, after_dma=load_weights_k)
            proj_x(1, [0], dma=True)
            se0.append(emit_se(0, 0, 0))
            se0.append(emit_se(0, 0, 1))
            proj_x(0, [1])
            se0.append(emit_se(0, 0, 2))
            proj_x(1, [1])
            se0.append(emit_se(0, 0, 3))
            proj_k(1, [0], dma=True, after_dma=load_weights_v)
            se0.append(emit_se(0, 0, 4))
            proj_k(0, [1])
            se0.append(emit_se(0, 0, 5))
            proj_x(2, [0, 1], dma=True)
            se0.append(emit_se(0, 0, 6))
            proj_k(2, [0], dma=True)
            se0.append(emit_se(0, 0, 7))
            proj_k(1, [1])
            se0.append(emit_se(0, 0, 8))
            proj_x(3, [0, 1], dma=True)
            se0.append(emit_se(0, 0, 9))
            proj_k(3, [0], dma=True, after_dma=load_weights_o)
            se0.append(emit_se(0, 0, 10))
            proj_k(2, [1])
            se0.append(emit_se(0, 0, 11))
            proj_k(3, [1])
            se0 += [emit_se(0, 0, j) for j in range(12, 16)]
            e[P[0]] = se0
            # pre-emit the next pair's first scores so ACT rolls straight on
            e[P[1]] = [emit_se(1, 0, j) for j in range(3)]

            # DVE-offloaded exp tiles: early-in-batch indices only, so the
            # Schraudolph ops never sit ahead of a norm chain in DVE's FIFO
            def se_fill(se, key, upto):
                if len(se) < min(upto, 16):
                    se.append(emit_se(*key, len(se),
                                      dve=(len(se) in (2, 5))))

            # ---- phase 0 (pair 0, window 0): split halves; v-projection
            # woven into half-a; pair-1 scores woven into half-b.
            t, sqw = P[0]
            exE = [a for a, _ in e[P[0]]]
            exO = [b for _, b in e[P[0]]]
            se = e[P[1]]
            atE = atp.tile([128, 512], F32, tag="at", name="atE")
            atO = atp.tile([128, 512], F32, tag="at", name="atO")
            for skc in range(16):
                if skc % 2 == 0:
                    proj_v(skc // 4, [skc % 4, skc % 4 + 1])
                mm_at(atE, exE[skc], 2 * t, skc, 0)
                mm_at(atO, exO[skc], 2 * t + 1, skc, 0)
            # half-a norms: DVE/Pool only, overlap half-b's PE work
            norm(t, sqw, 0, 0, atE)
            norm(t, sqw, 1, 0, atO)
            atE2 = atp.tile([128, 512], F32, tag="at", name="atE2")
            atO2 = atp.tile([128, 512], F32, tag="at", name="atO2")
            for skc in range(16):
                # se first: no norm dependency, covers the half-a norm chain
                se_fill(se, P[1], skc + 3)
                mm_at(atE2, exE[skc], 2 * t, skc, 1)
                mm_at(atO2, exO[skc], 2 * t + 1, skc, 1)
            norm(t, sqw, 0, 1, atE2)
            norm(t, sqw, 1, 1, atO2)
            while len(se) < 16:
                se_fill(se, P[1], 16)

            # ---- phases 1-3: fused sweeps.  Accumulator allocation and
            # sweep order are pp/stp-FIRST so the first matmuls of a sweep
            # never wait on the previous phase's atp-ring norms.
            for i in range(1, 4):
                t, sqw = P[i]
                exE = [a for a, _ in e[P[i]]]
                exO = [b for _, b in e[P[i]]]
                nxt = e.setdefault(P[i + 1], []) if i < 3 else None
                acc = stp if i == 3 else pp    # stp is free in the last phase
                acctag = "st" if i == 3 else "pp"
                atE2 = acc.tile([128, 512], F32, tag=acctag, name="atE2")
                atO2 = acc.tile([128, 512], F32, tag=acctag, name="atO2")
                atE = atp.tile([128, 512], F32, tag="at", name="atE")
                atO = atp.tile([128, 512], F32, tag="at", name="atO")
                for skc in range(16):
                    # hit the ring whose prior readers finished longest ago
                    # first, so a sweep's opening matmuls never stall
                    if i < 3:
                        mm_at(atE2, exE[skc], 2 * t, skc, 1)
                        mm_at(atO2, exO[skc], 2 * t + 1, skc, 1)
                        mm_at(atE, exE[skc], 2 * t, skc, 0)
                        mm_at(atO, exO[skc], 2 * t + 1, skc, 0)
                    else:
                        mm_at(atE, exE[skc], 2 * t, skc, 0)
                        mm_at(atO, exO[skc], 2 * t + 1, skc, 0)
                        mm_at(atE2, exE[skc], 2 * t, skc, 1)
                        mm_at(atO2, exO[skc], 2 * t + 1, skc, 1)
                    if nxt is not None:
                        se_fill(nxt, P[i + 1], skc)
                # half-a norms (atp accumulators); held-back score emissions
                # and norm-independent out_proj chunks cover the chains
                norm(t, sqw, 0, 0, atE)
                norm(t, sqw, 1, 0, atO)
                if nxt is not None:
                    se_fill(nxt, P[i + 1], 16)
                    se_fill(nxt, P[i + 1], 16)
                if i == 2:
                    # window-0 chunks deferred from phase 1: norm-independent
                    # PE work covering phase 2's norm chains
                    emit_out_proj(4, po_pool=(atp, "at"))
                    emit_out_proj(5, po_pool=(atp, "at"))
                if i == 3:
                    # deferred window-0 chunks: no phase-3 dependency, they
                    # cover the half-a norm chain; then (8-11) which only
                    # needs half-a of window 1
                    emit_out_proj(6, use_act=True)
                    emit_out_proj(7, use_act=True)
                    for sqc in range(8, 12):
                        emit_out_proj(sqc, use_act=True, po_pool=(atp, "at"))
                norm(t, sqw, 0, 1, atE2)
                norm(t, sqw, 1, 1, atO2)
                if nxt is not None:
                    while len(nxt) < 16:
                        se_fill(nxt, P[i + 1], 16)
                if i == 1:
                    for sqc in range(0, 4):
                        emit_out_proj(sqc, po_pool=(atp, "at"))
                elif i == 3:
                    for sqc in range(12, 16):
                        emit_out_proj(sqc, use_act=True)
            loop_ctx.__exit__(None, None, None)

    nc.compile()
    return nc


_NC = None


def _program():
    global _NC
    if _NC is None:
        _NC = build_program()
    return _NC


def _f32(a):
    return np.ascontiguousarray(np.asarray(a, dtype=np.float32))


def make_in_maps(inputs, context, Wq, bq, Wk, bk, Wv, bv, Wo, bo):
    inputs = np.asarray(inputs)
    context = np.asarray(context)
    Wq, bq, Wk, bk = (np.asarray(a) for a in (Wq, bq, Wk, bk))
    Wv, Wo = np.asarray(Wv), np.asarray(Wo)
    in_maps = []
    for core in range(NCORES):
        b, g = core // HG, core % HG
        sl = slice(DG * g, DG * (g + 1))
        in_maps.append({
            "xT": np.ascontiguousarray(inputs[b].T.astype(np.float16)),
            "cT": np.ascontiguousarray(context[b].T.astype(np.float16)),
            "wq": np.ascontiguousarray(Wq[:, sl].astype(np.float16)),
            "wk": np.ascontiguousarray(Wk[:, sl].astype(np.float16)),
            "wv": np.ascontiguousarray(Wv[:, sl].astype(np.float16)),
            "wo": np.ascontiguousarray(Wo[sl, :].astype(np.float16)),
            "bq": _f32(bq[sl]),
            "bk": _f32(bk[sl]),
        })
    return in_maps


def kernel(inputs, context, Wq, bq, Wk, bk, Wv, bv, Wo, bo):
    from concourse.bass_utils import run_bass_kernel_spmd

    nc = _program()
    in_maps = make_in_maps(inputs, context, Wq, bq, Wk, bk, Wv, bv, Wo, bo)
    res = run_bass_kernel_spmd(nc, in_maps, list(range(NCORES)))
    outs = [np.asarray(res.results[i]["out"]).astype(np.float32)
            for i in range(NCORES)]
    bv = _f32(bv)
    Wo = _f32(Wo)
    bo = _f32(bo)
    corr = (bv.astype(np.float64) @ Wo.astype(np.float64)
            + bo.astype(np.float64)).astype(np.float32)
    full = np.stack([
        outs[0] + outs[1] + outs[2] + outs[3],
        outs[4] + outs[5] + outs[6] + outs[7],
    ]) + corr
    return full.astype(np.float32)
